# revision 2
# baseline (speedup 1.0000x reference)
"""Causal MHA (B=4, L=2048, D=1024, H=16) on 8 NeuronCores — fused pipeline.

Sharding: core c -> (batch b = c//2, head-group g = c%2), 8 heads/core.
wq/wk/wv column-parallel, wo row-parallel; host sums the two half-group
partials per batch and adds wo_b.

Per-core kernel (single dataflow pipeline, no DRAM round-trips):
  For each 512-token slice n (4 slices):
    proj(n):  QT/KT = w.T @ x_n -> SBUF [128, m, L] f32r (dims on partitions)
              V_aug = x_n.T @ wv_aug -> SBUF vh [128keys, kb, h, 65] bf16
              (per head: 64 dims + ones column -> softmax denominator)
    att(n):   per head h, per 2-keyblock group: S.T[keys, q] on PE (f32r),
              exp on Act -> pt bf16, diag mask-mul on DVE (bf16 4x),
              AV transposed: psum[q, 65] += pt_chunk.T @ vh_kb (bf16, N=65)
              -> denominator lands as psum column 64; DVE per-partition
              reciprocal + tensor_scalar_mul -> ctx_t [q, 64] f32r
              PE-transpose ctx_t -> psum -> DVE copy -> ctxT [dims, q] bf16
    C(n):     out[t, :] += sum_c ctxT_c[:, t].T @ wo_c (bf16) -> DMA out
  proj(n+1) and C(n-1) matmuls are interleaved into att(n) as PE filler,
  paced so the Act engine's exp stream never stalls the PE.
"""

import numpy as np

import concourse.bacc as bacc
import concourse.bass as bass
import concourse.mybir as mybir
import concourse.tile as tile
from concourse.bass_utils import run_bass_kernel_spmd

F32 = mybir.dt.float32
F32R = mybir.dt.float32r
BF16 = mybir.dt.bfloat16

B, L, D, H, DK = 4, 2048, 1024, 16, 64
HD = 8              # heads per core
GW = 512            # head-group width
AUGW = HD * (DK + 1)  # 520
NCH = D // 128      # 8 contraction chunks
SL = 512            # token slice
NS = L // SL        # 4
NKB = L // 128      # 16

PE_NS = 1.0 / 2.4   # ns per PE cycle at full clock
ACT_NS = 1.0 / 1.2  # ns per Act cycle


def _build_nc(dbg=False):
    nc = bacc.Bacc("TRN2", target_bir_lowering=False, debug=False, num_devices=8)

    xq = nc.dram_tensor("xq", [D, L], F32R, kind="ExternalInput").ap()
    xk = nc.dram_tensor("xk", [D, L], F32R, kind="ExternalInput").ap()
    xv = nc.dram_tensor("xv", [D, L], F32R, kind="ExternalInput").ap()
    wq = nc.dram_tensor("wq", [D, GW], F32R, kind="ExternalInput").ap()
    wk = nc.dram_tensor("wk", [D, GW], F32R, kind="ExternalInput").ap()
    wv = nc.dram_tensor("wv", [D, AUGW], F32R, kind="ExternalInput").ap()
    wo = nc.dram_tensor("wo", [GW, D], BF16, kind="ExternalInput").ap()
    bq = nc.dram_tensor("bq", [128, 4], F32, kind="ExternalInput").ap()
    bk = nc.dram_tensor("bk", [128, 4], F32, kind="ExternalInput").ap()
    vb = nc.dram_tensor("vb", [AUGW], F32, kind="ExternalInput").ap()
    msk = nc.dram_tensor("msk", [128, 128], BF16, kind="ExternalInput").ap()
    idn = nc.dram_tensor("idn", [128, 128], BF16, kind="ExternalInput").ap()
    outp = nc.dram_tensor("outp", [L, D], F32, kind="ExternalOutput").ap()
    if dbg:
        qt_dbg = nc.dram_tensor("qt_dbg", [128, 4 * L], F32, kind="ExternalOutput").ap()
        kt_dbg = nc.dram_tensor("kt_dbg", [128, 4 * L], F32, kind="ExternalOutput").ap()
        vh_dbg = nc.dram_tensor("vh_dbg", [128, NKB * 520], F32,
                                kind="ExternalOutput").ap()
        ctx_dbg = nc.dram_tensor("ctx_dbg", [128, 4 * L], F32,
                                 kind="ExternalOutput").ap()

    with tile.TileContext(nc) as tc:
        with (
            tc.tile_pool(name="persist", bufs=1) as persist,
            tc.tile_pool(name="xin", bufs=16) as xinp,
            tc.tile_pool(name="pt", bufs=3) as ptp,
            tc.tile_pool(name="ctx", bufs=4) as ctxp,
            tc.tile_pool(name="ctxT", bufs=8) as ctxTp,
            tc.tile_pool(name="small", bufs=8) as smallp,
            tc.tile_pool(name="outs", bufs=3) as outsp,
            tc.tile_pool(name="psS", bufs=2, space="PSUM") as psS,
            tc.tile_pool(name="psAV", bufs=1, space="PSUM") as psAV,
            tc.tile_pool(name="psG", bufs=1, space="PSUM") as psG,
        ):
            # ---- persistent SBUF ----
            wq_s = persist.tile([128, NCH, GW], F32R, tag="wq")
            wk_s = persist.tile([128, NCH, GW], F32R, tag="wk")
            wv_s = persist.tile([128, NCH, AUGW], F32R, tag="wv")
            wo_s = persist.tile([128, 4, D], BF16, tag="wo")
            qt_s = persist.tile([128, 4, L], F32R, tag="qt")
            kt_s = persist.tile([128, 4, L], F32R, tag="kt")
            vh_s = persist.tile([128, NKB, 2, 260], BF16, tag="vh")
            bq_s = persist.tile([128, 4], F32, tag="bq")
            bk_s = persist.tile([128, 4], F32, tag="bk")
            vb_s = persist.tile([128, AUGW], F32, tag="vb")
            msk_s = persist.tile([128, 128], BF16, tag="msk")
            idn_s = persist.tile([128, 128], BF16, tag="idn")

            # weights on the gpsimd DMA queue (parallel with x/out traffic)
            for c in range(NCH):
                nc.gpsimd.dma_start(wq_s[:, c, :], wq[c * 128:(c + 1) * 128, :])
                nc.gpsimd.dma_start(wk_s[:, c, :], wk[c * 128:(c + 1) * 128, :])
            for c in range(NCH):
                nc.gpsimd.dma_start(wv_s[:, c, :], wv[c * 128:(c + 1) * 128, :])
            for c in range(4):
                nc.gpsimd.dma_start(wo_s[:, c, :], wo[c * 128:(c + 1) * 128, :])
            nc.gpsimd.dma_start(bq_s[:, :], bq[:, :])
            nc.gpsimd.dma_start(bk_s[:, :], bk[:, :])
            nc.gpsimd.dma_start(msk_s[:, :], msk[:, :])
            nc.gpsimd.dma_start(idn_s[:, :], idn[:, :])
            vb_bcast = bass.AP(tensor=vb.tensor, offset=vb.offset,
                               ap=[[0, 128], [1, AUGW]])
            nc.gpsimd.dma_start(vb_s[:, :], vb_bcast)

            # ---------- pacing counters (ns, at full clocks) ----------
            st = {"pe": 0.0, "act": 0.0}

            def mm(*args, **kw):
                out = args[0]
                st["pe"] += out.free_size() * PE_NS
                nc.tensor.matmul(*args, **kw)

            # ---------- projection / output-projection units ----------
            def emit_x_dmas(n, src, tag):
                tiles = []
                for c in range(NCH):
                    t = xinp.tile([128, SL], F32R, tag="x", name=f"x{tag}{n}_{c}")
                    nc.sync.dma_start(
                        t[:, :], src[c * 128:(c + 1) * 128, n * SL:(n + 1) * SL])
                    tiles.append(t)
                return tiles

            def emit_qk_unit(n, xt, w_s, dst, b_s, m):
                ps = psG.tile([128, 512], F32, tag="g", bufs=2, name=f"qk{n}_{m}")
                for c in range(NCH):
                    mm(ps[:, :], w_s[:, c, m * 128:(m + 1) * 128], xt[c][:, :],
                       start=(c == 0), stop=(c == NCH - 1))
                nc.vector.tensor_scalar_add(
                    dst[:, m, n * SL:(n + 1) * SL], ps[:, :], b_s[:, m:m + 1])

            def emit_v_unit(n, xt, tt, hf):
                ps = psG.tile([128, 512], F32, tag="g", bufs=2, name=f"v{n}_{tt}_{hf}")
                for c in range(NCH):
                    mm(ps[:, 0:260], xt[c][:, tt * 128:(tt + 1) * 128],
                       wv_s[:, c, hf * 260:(hf + 1) * 260],
                       start=(c == 0), stop=(c == NCH - 1))
                kb = n * 4 + tt
                nc.vector.tensor_add(
                    vh_s[:, kb, hf, :],
                    ps[:, 0:260], vb_s[:, hf * 260:(hf + 1) * 260])

            def emit_c_unit(n, tt, n2, ctxT_n):
                ps = psG.tile([128, 512], F32, tag="g", bufs=2, name=f"c{n}_{tt}_{n2}")
                for c in range(4):
                    mm(ps[:, :], ctxT_n[c][:, tt * 128:(tt + 1) * 128],
                       wo_s[:, c, n2 * 512:(n2 + 1) * 512],
                       start=(c == 0), stop=(c == 3))
                ot = outsp.tile([128, 512], F32, tag="ot", name=f"ot{n}_{tt}_{n2}")
                nc.vector.tensor_copy(ot[:, :], ps[:, :])
                nc.sync.dma_start(
                    outp[(n * 4 + tt) * 128:(n * 4 + tt + 1) * 128,
                         n2 * 512:(n2 + 1) * 512], ot[:, :])

            def proj_units(n, xts):
                units = []
                for m in range(4):
                    units.append(lambda n=n, m=m: emit_qk_unit(
                        n, xts["q"], wq_s, qt_s, bq_s, m))
                for m in range(4):
                    units.append(lambda n=n, m=m: emit_qk_unit(
                        n, xts["k"], wk_s, kt_s, bk_s, m))
                for tt in range(4):
                    for hf in range(2):
                        units.append(lambda n=n, tt=tt, hf=hf: emit_v_unit(
                            n, xts["v"], tt, hf))
                return units

            def c_units(n, ctxT_n):
                units = []
                for tt in range(4):
                    for n2 in range(2):
                        units.append(lambda n=n, tt=tt, n2=n2: emit_c_unit(
                            n, tt, n2, ctxT_n))
                return units

            # ---------- slice 0 projections (prologue, no filler) ----------
            xts0 = {}
            xts0["q"] = emit_x_dmas(0, xq, "q")
            for m in range(4):
                emit_qk_unit(0, xts0["q"], wq_s, qt_s, bq_s, m)
            xts0["k"] = emit_x_dmas(0, xk, "k")
            for m in range(4):
                emit_qk_unit(0, xts0["k"], wk_s, kt_s, bk_s, m)
            xts0["v"] = emit_x_dmas(0, xv, "v")
            for tt in range(4):
                for hf in range(2):
                    emit_v_unit(0, xts0["v"], tt, hf)

            # ---------- attention + pipeline ----------
            ctxT_all = {}   # n -> list of 4 chunk tiles

            for n in range(NS):
                # filler queue: C(n-1) first, then proj(n+1)
                fillers = []
                if n >= 1:
                    fillers += c_units(n - 1, ctxT_all[n - 1])
                nxt = {}
                if n + 1 < NS:
                    # lazy x DMAs: emit per-tensor right before first use
                    pending_dma = {"q": xq, "k": xk, "v": xv}
                    nxts = {}

                    def get_xt(which, n1=n + 1):
                        if which in pending_dma:
                            nxts[which] = emit_x_dmas(n1, pending_dma.pop(which),
                                                      which)
                        return nxts[which]

                    for m in range(4):
                        fillers.append(lambda n1=n + 1, m=m: emit_qk_unit(
                            n1, get_xt("q"), wq_s, qt_s, bq_s, m))
                    for m in range(4):
                        fillers.append(lambda n1=n + 1, m=m: emit_qk_unit(
                            n1, get_xt("k"), wk_s, kt_s, bk_s, m))
                    for tt in range(4):
                        for hf in range(2):
                            fillers.append(lambda n1=n + 1, tt=tt, hf=hf:
                                           emit_v_unit(n1, get_xt("v"), tt, hf))
                fill_i = [0]

                def pop_fillers(force_all=False):
                    while fill_i[0] < len(fillers) and (
                            force_all or st["pe"] < st["act"] + 500.0):
                        fillers[fill_i[0]]()
                        fill_i[0] += 1

                nkb = 4 * n + 4
                ngrp = nkb // 2
                ctxT_n = [ctxTp.tile([128, 512], BF16, tag="ctxT",
                                     name=f"ctxT{n}_{c}") for c in range(4)]
                ctxT_all[n] = ctxT_n
                psT_cur = [None]

                for h in range(HD):
                    mc = h // 2
                    po = (h % 2) * 64
                    avp = psAV.tile([128, 4, 128], F32, tag="av", name=f"av{n}_{h}")
                    pts = [None] * ngrp

                    def emit_s_exp(g, h=h, n=n, mc=mc, po=po, pts=None):
                        sps = psS.tile([128, 2, 512], F32, tag="s",
                                       name=f"s{n}_{h}_{g}")
                        pt = ptp.tile([128, 2, 512], BF16, tag="pt",
                                      name=f"pt{n}_{h}_{g}")
                        for i in range(2):
                            kb = 2 * g + i
                            col0 = max(0, kb * 128 - n * SL)
                            mm(sps[:, i, col0:],
                               kt_s[po:po + 64, mc, kb * 128:(kb + 1) * 128],
                               qt_s[po:po + 64, mc, n * SL + col0:(n + 1) * SL],
                               start=True, stop=True)
                        c0a = max(0, 2 * g * 128 - n * SL)
                        c0b = max(0, (2 * g + 1) * 128 - n * SL)
                        if c0a == c0b:
                            st["act"] += (2 * (512 - c0a)) * ACT_NS + 185.0
                            nc.scalar.activation(
                                pt[:, :, c0a:], sps[:, :, c0a:],
                                func=mybir.ActivationFunctionType.Exp)
                        else:
                            for i, c0 in ((0, c0a), (1, c0b)):
                                st["act"] += (512 - c0) * ACT_NS + 185.0
                                nc.scalar.activation(
                                    pt[:, i, c0:], sps[:, i, c0:],
                                    func=mybir.ActivationFunctionType.Exp)
                        for i in range(2):
                            kb = 2 * g + i
                            if kb >= 4 * n:  # diagonal block: causal mask
                                col0 = max(0, kb * 128 - n * SL)
                                nc.vector.tensor_mul(
                                    pt[:, i, col0:col0 + 128],
                                    pt[:, i, col0:col0 + 128], msk_s[:, :])
                        pts[g] = pt

                    def emit_av(g, h=h, n=n, avp=avp, pts=None):
                        # PSUM start=True lazily zero-marks the WHOLE bank, so
                        # only the first write into the bank may use it; kb0
                        # writes for qc>0 overwrite via the pending-zero flags.
                        pt = pts[g]
                        for i in range(2):
                            kb = 2 * g + i
                            for qc in range(4):
                                if kb > 4 * n + qc:
                                    continue
                                mm(avp[:, qc, 0:65],
                                   pt[:, i, qc * 128:(qc + 1) * 128],
                                   vh_s[:, kb, h // 4, (h % 4) * 65:(h % 4) * 65 + 65],
                                   start=(kb == 0 and qc == 0),
                                   stop=(kb == 4 * n + qc),
                                   skip_group_check=True)

                    emit_s_exp(0, pts=pts)
                    for g in range(1, ngrp):
                        emit_s_exp(g, pts=pts)
                        emit_av(g - 1, pts=pts)
                        pop_fillers()
                    emit_av(ngrp - 1, pts=pts)
                    pop_fillers()

                    # normalize: ctx_t[q, d] = av[q, d] / av[q, 64]
                    rcp = smallp.tile([128, 4, 1], F32, tag="rcp", name=f"rcp{n}_{h}")
                    nc.vector.reciprocal(rcp[:, :], avp[:, :, 64:65])
                    ctx_t = ctxp.tile([128, 4, DK], BF16, tag="ctx",
                                      name=f"ctx{n}_{h}")
                    for qc in range(4):
                        nc.vector.tensor_scalar_mul(
                            ctx_t[:, qc, :], avp[:, qc, 0:DK],
                            rcp[:, qc, :])

                    # transpose to dims-major; 2 heads share one psum tile
                    if h % 2 == 0:
                        psT_cur[0] = psG.tile([128, 512], BF16, tag="t",
                                              name=f"t{n}_{mc}")
                    psT = psT_cur[0]
                    for qc in range(4):
                        st["pe"] += 128 * PE_NS
                        nc.tensor.transpose(
                            psT[po:po + 64, qc * 128:(qc + 1) * 128],
                            ctx_t[:, qc, :], idn_s[:, :])
                    if h % 2 == 1:
                        nc.vector.tensor_copy(ctxT_n[mc][:, :], psT[:, :])

                pop_fillers(force_all=True)
                if dbg:
                    for c in range(4):
                        ct = outsp.tile([128, 512], F32, tag="ot",
                                        name=f"cdb{n}_{c}")
                        nc.vector.tensor_copy(ct[:, :], ctxT_n[c][:, :])
                        nc.sync.dma_start(
                            ctx_dbg[:, (n * 4 + c) * 512:(n * 4 + c + 1) * 512],
                            ct[:, :])

            # ---------- final output projection ----------
            for u in c_units(NS - 1, ctxT_all[NS - 1]):
                u()

            if dbg:
                nc.sync.dma_start(qt_dbg[:, :], qt_s[:, :, :].bitcast(F32))
                nc.sync.dma_start(kt_dbg[:, :], kt_s[:, :, :].bitcast(F32))
                for kb in range(NKB):
                    for hf in range(2):
                        vt = outsp.tile([128, 512], F32, tag="ot",
                                        name=f"vdb{kb}_{hf}")
                        nc.vector.tensor_copy(vt[:, 0:260], vh_s[:, kb, hf, :])
                        nc.sync.dma_start(
                            vh_dbg[:, kb * 520 + hf * 260:
                                   kb * 520 + (hf + 1) * 260], vt[:, 0:260])

    nc.compile()
    return nc


_NC = None
LAST_RESULTS = None


def kernel(**inputs):
    global _NC, LAST_RESULTS
    import os
    import ml_dtypes
    if _NC is None:
        _NC = _build_nc()

    f = lambda a: np.asarray(a, dtype=np.float32)
    q, k, v = f(inputs["q"]), f(inputs["k"]), f(inputs["v"])
    wq_w, wq_b = f(inputs["wq_w"]), f(inputs["wq_b"])
    wk_w, wk_b = f(inputs["wk_w"]), f(inputs["wk_b"])
    wv_w, wv_b = f(inputs["wv_w"]), f(inputs["wv_b"])
    wo_w, wo_b = f(inputs["wo_w"]), f(inputs["wo_b"])

    bf = ml_dtypes.bfloat16
    msk = np.ascontiguousarray(
        (np.arange(128)[None, :] >= np.arange(128)[:, None])).astype(bf)
    idn = np.eye(128).astype(bf)

    gmaps = []
    for g in range(2):
        sl = slice(g * GW, (g + 1) * GW)
        wqT = np.ascontiguousarray((wq_w[sl] * 0.125).T)
        wkT = np.ascontiguousarray(wk_w[sl].T)
        wvT = np.zeros((D, AUGW), np.float32)
        vbias = np.zeros((AUGW,), np.float32)
        for h in range(HD):
            wvT[:, h * 65:h * 65 + 64] = wv_w[g * GW + h * 64:
                                              g * GW + (h + 1) * 64].T
            vbias[h * 65:h * 65 + 64] = wv_b[g * GW + h * 64:
                                             g * GW + (h + 1) * 64]
            vbias[h * 65 + 64] = 1.0
        woT = np.ascontiguousarray(wo_w[:, sl].T).astype(bf)
        bqT = np.ascontiguousarray((wq_b[sl] * 0.125).reshape(4, 128).T)
        bkT = np.ascontiguousarray(wk_b[sl].reshape(4, 128).T)
        gmaps.append(dict(wq=wqT, wk=wkT, wv=wvT, wo=woT, bq=bqT, bk=bkT,
                          vb=vbias, msk=msk, idn=idn))

    bmaps = []
    for b in range(B):
        bmaps.append(dict(
            xq=np.ascontiguousarray(q[b].T),
            xk=np.ascontiguousarray(k[b].T),
            xv=np.ascontiguousarray(v[b].T)))

    in_maps = [dict(**bmaps[c // 2], **gmaps[c % 2]) for c in range(8)]

    trace = bool(int(os.environ.get("KERNEL_TRACE", "0")))
    res = run_bass_kernel_spmd(_NC, in_maps, list(range(8)), trace=trace)
    LAST_RESULTS = res

    out = np.empty((B, L, D), np.float32)
    for b in range(B):
        out[b] = (res.results[2 * b]["outp"] + res.results[2 * b + 1]["outp"]
                  + wo_b[None, :])
    return out


# revision 3
# speedup vs baseline: 1.1422x; 1.1422x over previous
"""Causal MHA (B=4, L=2048, D=1024, H=16) on 8 NeuronCores — fused pipeline.

Sharding: core c -> (batch b = c//2, head-group g = c%2), 8 heads/core.
wq/wk/wv column-parallel, wo row-parallel; host sums the two half-group
partials per batch and adds wo_b.

Per-core kernel (single dataflow pipeline, no DRAM round-trips):
  For each 512-token slice n (4 slices):
    proj(n):  QT/KT = w.T @ x_n -> SBUF [128, m, L] f32r (dims on partitions)
              V_aug = x_n.T @ wv_aug -> SBUF vh [128keys, kb, h, 65] bf16
              (per head: 64 dims + ones column -> softmax denominator)
    att(n):   per head h, per 2-keyblock group: S.T[keys, q] on PE (f32r),
              exp on Act -> pt bf16, diag mask-mul on DVE (bf16 4x),
              AV transposed: psum[q, 65] += pt_chunk.T @ vh_kb (bf16, N=65)
              -> denominator lands as psum column 64; DVE per-partition
              reciprocal + tensor_scalar_mul -> ctx_t [q, 64] f32r
              PE-transpose ctx_t -> psum -> DVE copy -> ctxT [dims, q] bf16
    C(n):     out[t, :] += sum_c ctxT_c[:, t].T @ wo_c (bf16) -> DMA out
  proj(n+1) and C(n-1) matmuls are interleaved into att(n) as PE filler,
  paced so the Act engine's exp stream never stalls the PE.
"""

import numpy as np
import os as _os

import concourse.bacc as bacc
import concourse.bass as bass
import concourse.mybir as mybir
import concourse.tile as tile
from concourse.bass_utils import run_bass_kernel_spmd

F32 = mybir.dt.float32
F32R = mybir.dt.float32r
BF16 = mybir.dt.bfloat16

B, L, D, H, DK = 4, 2048, 1024, 16, 64
HD = 8              # heads per core
GW = 512            # head-group width
AUGW = HD * (DK + 1)  # 520
NCH = D // 128      # 8 contraction chunks
SL = 512            # token slice
NS = L // SL        # 4
NKB = L // 128      # 16

PE_NS = 1.0 / 2.4   # ns per PE cycle at full clock
ACT_NS = 1.0 / 1.2  # ns per Act cycle


def _build_nc(dbg=False):
    nc = bacc.Bacc("TRN2", target_bir_lowering=False, debug=False, num_devices=8)

    xq = nc.dram_tensor("xq", [D, L], BF16, kind="ExternalInput").ap()
    xk = nc.dram_tensor("xk", [D, L], BF16, kind="ExternalInput").ap()
    xv = nc.dram_tensor("xv", [D, L], BF16, kind="ExternalInput").ap()
    wq = nc.dram_tensor("wq", [D, GW], BF16, kind="ExternalInput").ap()
    wk = nc.dram_tensor("wk", [D, GW], BF16, kind="ExternalInput").ap()
    wv = nc.dram_tensor("wv", [D, AUGW], BF16, kind="ExternalInput").ap()
    wo = nc.dram_tensor("wo", [GW, D], BF16, kind="ExternalInput").ap()
    bq = nc.dram_tensor("bq", [128, 4], F32, kind="ExternalInput").ap()
    bk = nc.dram_tensor("bk", [128, 4], F32, kind="ExternalInput").ap()
    vb = nc.dram_tensor("vb", [AUGW], F32, kind="ExternalInput").ap()
    msk = nc.dram_tensor("msk", [128, 128], BF16, kind="ExternalInput").ap()
    idn = nc.dram_tensor("idn", [128, 128], BF16, kind="ExternalInput").ap()
    outp = nc.dram_tensor("outp", [L, D], F32, kind="ExternalOutput").ap()
    if dbg:
        qt_dbg = nc.dram_tensor("qt_dbg", [128, 4 * L], F32, kind="ExternalOutput").ap()
        kt_dbg = nc.dram_tensor("kt_dbg", [128, 4 * L], F32, kind="ExternalOutput").ap()
        vh_dbg = nc.dram_tensor("vh_dbg", [128, NKB * 520], F32,
                                kind="ExternalOutput").ap()
        ctx_dbg = nc.dram_tensor("ctx_dbg", [128, 4 * L], F32,
                                 kind="ExternalOutput").ap()

    with tile.TileContext(nc) as tc:
        with (
            tc.tile_pool(name="persist", bufs=1) as persist,
            tc.tile_pool(name="xin", bufs=12) as xinp,
            tc.tile_pool(name="pt", bufs=4) as ptp,
            tc.tile_pool(name="ctx", bufs=4) as ctxp,
            tc.tile_pool(name="ctxT", bufs=8) as ctxTp,
            tc.tile_pool(name="small", bufs=8) as smallp,
            tc.tile_pool(name="outs", bufs=3) as outsp,
            tc.tile_pool(name="psS", bufs=2, space="PSUM") as psS,
            tc.tile_pool(name="psAV", bufs=1, space="PSUM") as psAV,
            tc.tile_pool(name="psG", bufs=1, space="PSUM") as psG,
        ):
            # ---- persistent SBUF ----
            wq_s = persist.tile([128, NCH, GW], BF16, tag="wq")
            wk_s = persist.tile([128, NCH, GW], BF16, tag="wk")
            wv_s = persist.tile([128, NCH, AUGW], BF16, tag="wv")
            wo_s = persist.tile([128, 4, D], BF16, tag="wo")
            qt_s = persist.tile([128, 4, L], F32R, tag="qt")
            kt_s = persist.tile([128, 4, L], F32R, tag="kt")
            vh_s = persist.tile([128, NKB, 2, 260], BF16, tag="vh")
            bq_s = persist.tile([128, 4], F32, tag="bq")
            bk_s = persist.tile([128, 4], F32, tag="bk")
            vb_s = persist.tile([128, AUGW], F32, tag="vb")
            msk_s = persist.tile([128, 128], BF16, tag="msk")
            idn_s = persist.tile([128, 128], BF16, tag="idn")

            # Weight DMAs ride the gpsimd queue; the global DMA device is
            # shared, so emission order approximates service order.  Slice-0
            # x chunks are emitted interleaved (in emit_x_dmas below).
            def emit_w_dmas(which):
                if which == "first":
                    nc.sync.dma_start(bq_s[:, :], bq[:, :])
                    nc.sync.dma_start(bk_s[:, :], bk[:, :])
                    nc.sync.dma_start(msk_s[:, :], msk[:, :])
                    nc.sync.dma_start(idn_s[:, :], idn[:, :])
                    vb_bcast = bass.AP(tensor=vb.tensor, offset=vb.offset,
                                       ap=[[0, 128], [1, AUGW]])
                    nc.sync.dma_start(vb_s[:, :], vb_bcast)
                elif which in ("q", "k", "v"):
                    w_s, w_d = {"q": (wq_s, wq), "k": (wk_s, wk),
                                "v": (wv_s, wv)}[which]
                    for hh in range(2):
                        nc.sync.dma_start(
                            w_s[:, hh * 4:(hh + 1) * 4, :],
                            w_d[hh * 512:(hh + 1) * 512, :].rearrange(
                                "(c p) q -> p c q", p=128))
                else:
                    nc.sync.dma_start(
                        wo_s[:, :, :], wo.rearrange("(c p) q -> p c q", p=128))

            # ---------- pacing counters (ns, at full clocks) ----------
            st = {"pe": 0.0, "act": 0.0}

            def mm(*args, **kw):
                out = args[0]
                st["pe"] += out.free_size() * PE_NS
                nc.tensor.matmul(*args, **kw)

            # ---------- projection / output-projection units ----------
            def emit_x_dmas(n, src, tag):
                halves = []
                for hh in range(2):
                    t = xinp.tile([128, 4, SL], BF16, tag="x",
                                  name=f"x{tag}{n}_{hh}")
                    nc.sync.dma_start(
                        t[:, :, :],
                        src[hh * 512:(hh + 1) * 512,
                            n * SL:(n + 1) * SL].rearrange(
                                "(c p) q -> p c q", p=128))
                    halves.append(t)
                return halves

            def emit_qk_unit(n, xt, w_s, dst, b_s, m, hh, psh):
                hhs = (0, 1) if hh is None else (hh,)
                if hhs[0] == 0:
                    psh[m] = psG.tile([128, 512], F32, tag="g", bufs=2,
                                      name=f"qk{n}_{m}")
                ps = psh[m]
                for h2 in hhs:
                    for c in range(4):
                        mm(ps[:, :], w_s[:, h2 * 4 + c, m * 128:(m + 1) * 128],
                           xt[h2][:, c, :],
                           start=(h2 == 0 and c == 0),
                           stop=(h2 == 1 and c == 3))
                if hhs[-1] == 1:
                    del psh[m]
                    nc.vector.tensor_scalar_add(
                        dst[:, m, n * SL:(n + 1) * SL], ps[:, :],
                        b_s[:, m:m + 1])

            def emit_v_unit(n, xt, tt, hf, hh, psh):
                hhs = (0, 1) if hh is None else (hh,)
                if hhs[0] == 0:
                    psh[(tt, hf)] = psG.tile([128, 512], F32, tag="g", bufs=2,
                                             name=f"v{n}_{tt}_{hf}")
                ps = psh[(tt, hf)]
                for h2 in hhs:
                    for c in range(4):
                        mm(ps[:, 0:260], xt[h2][:, c, tt * 128:(tt + 1) * 128],
                           wv_s[:, h2 * 4 + c, hf * 260:(hf + 1) * 260],
                           start=(h2 == 0 and c == 0),
                           stop=(h2 == 1 and c == 3))
                if hhs[-1] == 1:
                    del psh[(tt, hf)]
                    kb = n * 4 + tt
                    nc.vector.tensor_add(
                        vh_s[:, kb, hf, :],
                        ps[:, 0:260], vb_s[:, hf * 260:(hf + 1) * 260])

            def emit_c_unit(n, tt, n2, ctxT_n):
                ps = psG.tile([128, 512], F32, tag="g", bufs=2, name=f"c{n}_{tt}_{n2}")
                for c in range(4):
                    mm(ps[:, :], ctxT_n[c][:, tt * 128:(tt + 1) * 128],
                       wo_s[:, c, n2 * 512:(n2 + 1) * 512],
                       start=(c == 0), stop=(c == 3))
                ot = outsp.tile([128, 512], F32, tag="ot", name=f"ot{n}_{tt}_{n2}")
                nc.vector.tensor_copy(ot[:, :], ps[:, :])
                nc.sync.dma_start(
                    outp[(n * 4 + tt) * 128:(n * 4 + tt + 1) * 128,
                         n2 * 512:(n2 + 1) * 512], ot[:, :])


            def c_units(n, ctxT_n):
                units = []
                for tt in range(4):
                    for n2 in range(2):
                        units.append(lambda n=n, tt=tt, n2=n2: emit_c_unit(
                            n, tt, n2, ctxT_n))
                return units

            # ---------- slice 0 projections (prologue, no filler) ----------
            xts0 = {}
            emit_w_dmas("q")
            xts0["q"] = emit_x_dmas(0, xq, "q")
            emit_w_dmas("k")
            xts0["k"] = emit_x_dmas(0, xk, "k")
            emit_w_dmas("first")
            psh0 = {}
            for m in range(4):
                for hh in range(2):
                    emit_qk_unit(0, xts0["q"], wq_s, qt_s, bq_s, m, hh, psh0)
            emit_w_dmas("v")
            xts0["v"] = emit_x_dmas(0, xv, "v")
            for m in range(4):
                for hh in range(2):
                    emit_qk_unit(0, xts0["k"], wk_s, kt_s, bk_s, m, hh, psh0)
            emit_w_dmas("rest")
            for tt in range(4):
                for hf in range(2):
                    for hh in range(2):
                        emit_v_unit(0, xts0["v"], tt, hf, hh, psh0)

            # ---------- attention + pipeline ----------
            ctxT_all = {}   # n -> list of 4 chunk tiles

            for n in range(NS):
                # filler queue: C(n-1) first, then proj(n+1)
                fillers = []
                if n >= 1:
                    fillers += c_units(n - 1, ctxT_all[n - 1])
                if n + 1 < NS:
                    # lazy x DMAs: emit per-tensor right before first use
                    pending_dma = {"q": xq, "k": xk, "v": xv}
                    nxts = {}
                    pshn = {}

                    def get_xt(which, n1=n + 1):
                        if which in pending_dma:
                            nxts[which] = emit_x_dmas(n1, pending_dma.pop(which),
                                                      which)
                        return nxts[which]

                    for m in range(4):
                        fillers.append(
                            lambda n1=n + 1, m=m: emit_qk_unit(
                                n1, get_xt("q"), wq_s, qt_s, bq_s, m, None,
                                pshn))
                    for m in range(4):
                        fillers.append(
                            lambda n1=n + 1, m=m: emit_qk_unit(
                                n1, get_xt("k"), wk_s, kt_s, bk_s, m, None,
                                pshn))
                    for tt in range(4):
                        for hf in range(2):
                            fillers.append(
                                lambda n1=n + 1, tt=tt, hf=hf:
                                emit_v_unit(n1, get_xt("v"), tt, hf, None,
                                            pshn))
                fill_i = [0]

                def pop_fillers(force_all=False):
                    while fill_i[0] < len(fillers) and (
                            force_all or st["pe"] < st["act"] + float(_os.environ.get("K_MARGIN", 2500))):
                        fillers[fill_i[0]]()
                        fill_i[0] += 1

                nkb = 4 * n + 4
                ngrp = nkb // 2
                ctxT_n = [ctxTp.tile([128, 512], BF16, tag="ctxT",
                                     name=f"ctxT{n}_{c}") for c in range(4)]
                ctxT_all[n] = ctxT_n
                psT_cur = [None]

                def emit_s_exp(h, g, pts):
                    # Both banks of a group share the group's column base so a
                    # single 2-bank exp covers them (the extra computed scores
                    # land in q-chunks the AV stage never reads).
                    mc, po = h // 2, (h % 2) * 64
                    sps = psS.tile([128, 2, 512], F32, tag="s",
                                   name=f"s{n}_{h}_{g}")
                    pt = ptp.tile([128, 2, 512], BF16, tag="pt",
                                  name=f"pt{n}_{h}_{g}")
                    c0a = max(0, 2 * g * 128 - n * SL)
                    for i in range(2):
                        kb = 2 * g + i
                        mm(sps[:, i, c0a:],
                           kt_s[po:po + 64, mc, kb * 128:(kb + 1) * 128],
                           qt_s[po:po + 64, mc, n * SL + c0a:(n + 1) * SL],
                           start=True, stop=True)
                    st["act"] += (2 * (512 - c0a)) * ACT_NS + 185.0
                    nc.scalar.activation(
                        pt[:, :, c0a:], sps[:, :, c0a:],
                        func=mybir.ActivationFunctionType.Exp)
                    pts[(h, g)] = pt

                def emit_av(h, g, avp, pts, first_grp, last_grp):
                    # PSUM start=True lazily zero-marks the WHOLE bank, so
                    # only the first emitted write into the bank may use it;
                    # later first-writes per region overwrite via the
                    # pending-zero flags.  Accumulation order over kb is free.
                    pt = pts.pop((h, g))
                    for i in range(2):
                        kb = 2 * g + i
                        if kb >= 4 * n:  # diagonal block: causal mask
                            col0 = max(0, kb * 128 - n * SL)
                            (nc.gpsimd if _os.environ.get("K_MASKPOOL")
                             else nc.vector).tensor_mul(
                                pt[:, i, col0:col0 + 128],
                                pt[:, i, col0:col0 + 128], msk_s[:, :])
                    started = [not (g == first_grp)]
                    for i in range(2):
                        kb = 2 * g + i
                        for qc in range(4):
                            if kb > 4 * n + qc:
                                continue
                            last = (g == last_grp) and (
                                kb == min(1, 4 * n + qc))
                            mm(avp[:, qc, 0:65],
                               pt[:, i, qc * 128:(qc + 1) * 128],
                               vh_s[:, kb, h // 4, (h % 4) * 65:(h % 4) * 65 + 65],
                               start=not started[0],
                               stop=last,
                               skip_group_check=True)
                            started[0] = True

                def emit_tail(h, avp):
                    # normalize: ctx_t[q, d] = av[q, d] / av[q, 64]
                    mc, po = h // 2, (h % 2) * 64
                    rcp = smallp.tile([128, 4, 1], F32, tag="rcp",
                                      name=f"rcp{n}_{h}")
                    nc.vector.reciprocal(rcp[:, :], avp[:, :, 64:65])
                    ctx_t = ctxp.tile([128, 4, DK], BF16, tag="ctx",
                                      name=f"ctx{n}_{h}")
                    for qc in range(4):
                        nc.vector.tensor_scalar_mul(
                            ctx_t[:, qc, :], avp[:, qc, 0:DK], rcp[:, qc, :])
                    # transpose to dims-major; 2 heads share one psum tile
                    if h % 2 == 0:
                        psT_cur[0] = psG.tile([128, 512], BF16, tag="t",
                                              name=f"t{n}_{mc}")
                    psT = psT_cur[0]
                    for qc in range(4):
                        st["pe"] += 128 * PE_NS
                        nc.tensor.transpose(
                            psT[po:po + 64, qc * 128:(qc + 1) * 128],
                            ctx_t[:, qc, :], idn_s[:, :])
                    if h % 2 == 1:
                        nc.vector.tensor_copy(ctxT_n[mc][:, :], psT[:, :])

                # flat (head, group) pipeline: AV lags S/exp by one item so
                # head boundaries don't bunch the Act queue against psS WARs
                gorder = list(range(ngrp - 1, -1, -1))  # diag groups first
                items = [(h, g) for h in range(HD) for g in gorder]
                pts = {}
                avps = {}
                prev = None
                for (h, g) in items:
                    if g == gorder[0]:
                        avps[h] = psAV.tile([128, 4, 128], F32, tag="av",
                                            name=f"av{n}_{h}")
                    emit_s_exp(h, g, pts)
                    if prev is not None:
                        ph, pg = prev
                        emit_av(ph, pg, avps[ph], pts, gorder[0], gorder[-1])
                        if pg == gorder[-1]:
                            emit_tail(ph, avps.pop(ph))
                    pop_fillers()
                    prev = (h, g)
                ph, pg = prev
                emit_av(ph, pg, avps[ph], pts, gorder[0], gorder[-1])
                emit_tail(ph, avps.pop(ph))

                pop_fillers(force_all=True)
                if dbg:
                    for c in range(4):
                        ct = outsp.tile([128, 512], F32, tag="ot",
                                        name=f"cdb{n}_{c}")
                        nc.vector.tensor_copy(ct[:, :], ctxT_n[c][:, :])
                        nc.sync.dma_start(
                            ctx_dbg[:, (n * 4 + c) * 512:(n * 4 + c + 1) * 512],
                            ct[:, :])

            # ---------- final output projection ----------
            for u in c_units(NS - 1, ctxT_all[NS - 1]):
                u()

            if dbg:
                nc.sync.dma_start(qt_dbg[:, :], qt_s[:, :, :].bitcast(F32))
                nc.sync.dma_start(kt_dbg[:, :], kt_s[:, :, :].bitcast(F32))
                for kb in range(NKB):
                    for hf in range(2):
                        vt = outsp.tile([128, 512], F32, tag="ot",
                                        name=f"vdb{kb}_{hf}")
                        nc.vector.tensor_copy(vt[:, 0:260], vh_s[:, kb, hf, :])
                        nc.sync.dma_start(
                            vh_dbg[:, kb * 520 + hf * 260:
                                   kb * 520 + (hf + 1) * 260], vt[:, 0:260])

    nc.compile()
    return nc


_NC = None
LAST_RESULTS = None


def kernel(**inputs):
    global _NC, LAST_RESULTS
    import os
    import ml_dtypes
    if _NC is None:
        _NC = _build_nc()

    f = lambda a: np.asarray(a, dtype=np.float32)
    q, k, v = f(inputs["q"]), f(inputs["k"]), f(inputs["v"])
    wq_w, wq_b = f(inputs["wq_w"]), f(inputs["wq_b"])
    wk_w, wk_b = f(inputs["wk_w"]), f(inputs["wk_b"])
    wv_w, wv_b = f(inputs["wv_w"]), f(inputs["wv_b"])
    wo_w, wo_b = f(inputs["wo_w"]), f(inputs["wo_b"])

    bf = ml_dtypes.bfloat16
    msk = np.ascontiguousarray(
        (np.arange(128)[None, :] >= np.arange(128)[:, None])).astype(bf)
    idn = np.eye(128).astype(bf)

    gmaps = []
    for g in range(2):
        sl = slice(g * GW, (g + 1) * GW)
        wqT = np.ascontiguousarray((wq_w[sl] * 0.125).T).astype(bf)
        wkT = np.ascontiguousarray(wk_w[sl].T).astype(bf)
        wvT = np.zeros((D, AUGW), np.float32)
        vbias = np.zeros((AUGW,), np.float32)
        for h in range(HD):
            wvT[:, h * 65:h * 65 + 64] = wv_w[g * GW + h * 64:
                                              g * GW + (h + 1) * 64].T
            vbias[h * 65:h * 65 + 64] = wv_b[g * GW + h * 64:
                                             g * GW + (h + 1) * 64]
            vbias[h * 65 + 64] = 1.0
        woT = np.ascontiguousarray(wo_w[:, sl].T).astype(bf)
        bqT = np.ascontiguousarray((wq_b[sl] * 0.125).reshape(4, 128).T)
        bkT = np.ascontiguousarray(wk_b[sl].reshape(4, 128).T)
        gmaps.append(dict(wq=wqT, wk=wkT, wv=wvT.astype(bf), wo=woT, bq=bqT, bk=bkT,
                          vb=vbias, msk=msk, idn=idn))

    bmaps = []
    for b in range(B):
        bmaps.append(dict(
            xq=np.ascontiguousarray(q[b].T).astype(bf),
            xk=np.ascontiguousarray(k[b].T).astype(bf),
            xv=np.ascontiguousarray(v[b].T).astype(bf)))

    in_maps = [dict(**bmaps[c // 2], **gmaps[c % 2]) for c in range(8)]

    trace = bool(int(os.environ.get("KERNEL_TRACE", "0")))
    res = run_bass_kernel_spmd(_NC, in_maps, list(range(8)), trace=trace)
    LAST_RESULTS = res

    out = np.empty((B, L, D), np.float32)
    for b in range(B):
        out[b] = (res.results[2 * b]["outp"] + res.results[2 * b + 1]["outp"]
                  + wo_b[None, :])
    return out


# revision 4
# speedup vs baseline: 1.1614x; 1.0168x over previous
"""Causal MHA (B=4, L=2048, D=1024, H=16) on 8 NeuronCores — fused pipeline.

Sharding: core c -> (batch b = c//2, head-group g = c%2), 8 heads/core.
wq/wk/wv column-parallel, wo row-parallel; host sums the two half-group
partials per batch and adds wo_b.

Per-core kernel (single dataflow pipeline, no DRAM round-trips):
  For each 512-token slice n (4 slices):
    proj(n):  QT/KT = w.T @ x_n -> SBUF [128, m, L] f32r (dims on partitions)
              V_aug = x_n.T @ wv_aug -> SBUF vh [128keys, kb, h, 65] bf16
              (per head: 64 dims + ones column -> softmax denominator)
    att(n):   per head h, per 2-keyblock group: S.T[keys, q] on PE (f32r),
              exp on Act -> pt bf16, diag mask-mul on DVE (bf16 4x),
              AV transposed: psum[q, 65] += pt_chunk.T @ vh_kb (bf16, N=65)
              -> denominator lands as psum column 64; DVE per-partition
              reciprocal + tensor_scalar_mul -> ctx_t [q, 64] f32r
              PE-transpose ctx_t -> psum -> DVE copy -> ctxT [dims, q] bf16
    C(n):     out[t, :] += sum_c ctxT_c[:, t].T @ wo_c (bf16) -> DMA out
  proj(n+1) and C(n-1) matmuls are interleaved into att(n) as PE filler,
  paced so the Act engine's exp stream never stalls the PE.
"""

import numpy as np
import os as _os

import concourse.bacc as bacc
import concourse.bass as bass
import concourse.mybir as mybir
import concourse.tile as tile
from concourse.bass_utils import run_bass_kernel_spmd

F32 = mybir.dt.float32
F32R = mybir.dt.float32r
BF16 = mybir.dt.bfloat16

B, L, D, H, DK = 4, 2048, 1024, 16, 64
HD = 8              # heads per core
GW = 512            # head-group width
AUGW = HD * (DK + 1)  # 520
NCH = D // 128      # 8 contraction chunks
SL = 512            # token slice
NS = L // SL        # 4
NKB = L // 128      # 16

PE_NS = 1.0 / 2.4   # ns per PE cycle at full clock
ACT_NS = 1.0 / 1.2  # ns per Act cycle


def _build_nc(dbg=False):
    nc = bacc.Bacc("TRN2", target_bir_lowering=False, debug=False, num_devices=8)

    xq = nc.dram_tensor("xq", [D, L], BF16, kind="ExternalInput").ap()
    xk = nc.dram_tensor("xk", [D, L], BF16, kind="ExternalInput").ap()
    xv = nc.dram_tensor("xv", [D, L], BF16, kind="ExternalInput").ap()
    wq = nc.dram_tensor("wq", [D, GW], BF16, kind="ExternalInput").ap()
    wk = nc.dram_tensor("wk", [D, GW], BF16, kind="ExternalInput").ap()
    wv = nc.dram_tensor("wv", [D, AUGW], BF16, kind="ExternalInput").ap()
    wo = nc.dram_tensor("wo", [GW, D], BF16, kind="ExternalInput").ap()
    bq = nc.dram_tensor("bq", [128, 4], F32, kind="ExternalInput").ap()
    bk = nc.dram_tensor("bk", [128, 4], F32, kind="ExternalInput").ap()
    vb = nc.dram_tensor("vb", [AUGW], F32, kind="ExternalInput").ap()
    msk = nc.dram_tensor("msk", [128, 128], BF16, kind="ExternalInput").ap()
    idn = nc.dram_tensor("idn", [128, 128], BF16, kind="ExternalInput").ap()
    outp = nc.dram_tensor("outp", [L, D], BF16, kind="ExternalOutput").ap()
    if dbg:
        qt_dbg = nc.dram_tensor("qt_dbg", [128, 4 * L], F32, kind="ExternalOutput").ap()
        kt_dbg = nc.dram_tensor("kt_dbg", [128, 4 * L], F32, kind="ExternalOutput").ap()
        vh_dbg = nc.dram_tensor("vh_dbg", [128, NKB * 520], F32,
                                kind="ExternalOutput").ap()
        ctx_dbg = nc.dram_tensor("ctx_dbg", [128, 4 * L], F32,
                                 kind="ExternalOutput").ap()

    with tile.TileContext(nc) as tc:
        with (
            tc.tile_pool(name="persist", bufs=1) as persist,
            tc.tile_pool(name="xin", bufs=12) as xinp,
            tc.tile_pool(name="pt", bufs=5) as ptp,
            tc.tile_pool(name="ctx", bufs=4) as ctxp,
            tc.tile_pool(name="ctxT", bufs=8) as ctxTp,
            tc.tile_pool(name="small", bufs=8) as smallp,
            tc.tile_pool(name="outs", bufs=3) as outsp,
            tc.tile_pool(name="psS", bufs=2, space="PSUM") as psS,
            tc.tile_pool(name="psAV", bufs=1, space="PSUM") as psAV,
            tc.tile_pool(name="psG", bufs=1, space="PSUM") as psG,
        ):
            # ---- persistent SBUF ----
            wq_s = persist.tile([128, NCH, GW], BF16, tag="wq")
            wk_s = persist.tile([128, NCH, GW], BF16, tag="wk")
            wv_s = persist.tile([128, NCH, AUGW], BF16, tag="wv")
            wo_s = persist.tile([128, 4, D], BF16, tag="wo")
            qt_s = persist.tile([128, 4, L], F32R, tag="qt")
            kt_s = persist.tile([128, 4, L], F32R, tag="kt")
            vh_s = persist.tile([128, NKB, 2, 260], BF16, tag="vh")
            bq_s = persist.tile([128, 4], F32, tag="bq")
            bk_s = persist.tile([128, 4], F32, tag="bk")
            vb_s = persist.tile([128, AUGW], F32, tag="vb")
            msk_s = persist.tile([128, 128], BF16, tag="msk")
            idn_s = persist.tile([128, 128], BF16, tag="idn")

            # Weight DMAs ride the gpsimd queue; the global DMA device is
            # shared, so emission order approximates service order.  Slice-0
            # x chunks are emitted interleaved (in emit_x_dmas below).
            def emit_w_dmas(which):
                if which == "first":
                    nc.sync.dma_start(bq_s[:, :], bq[:, :])
                    nc.sync.dma_start(bk_s[:, :], bk[:, :])
                    nc.sync.dma_start(msk_s[:, :], msk[:, :])
                    nc.sync.dma_start(idn_s[:, :], idn[:, :])
                    vb_bcast = bass.AP(tensor=vb.tensor, offset=vb.offset,
                                       ap=[[0, 128], [1, AUGW]])
                    nc.sync.dma_start(vb_s[:, :], vb_bcast)
                elif which in ("q", "k", "v"):
                    w_s, w_d = {"q": (wq_s, wq), "k": (wk_s, wk),
                                "v": (wv_s, wv)}[which]
                    for hh in range(2):
                        nc.sync.dma_start(
                            w_s[:, hh * 4:(hh + 1) * 4, :],
                            w_d[hh * 512:(hh + 1) * 512, :].rearrange(
                                "(c p) q -> p c q", p=128))
                else:
                    nc.sync.dma_start(
                        wo_s[:, :, :], wo.rearrange("(c p) q -> p c q", p=128))

            # ---------- pacing counters (ns, at full clocks) ----------
            st = {"pe": 0.0, "act": 0.0}

            def mm(*args, **kw):
                out = args[0]
                st["pe"] += out.free_size() * PE_NS
                nc.tensor.matmul(*args, **kw)

            # ---------- projection / output-projection units ----------
            def emit_x_dmas(n, src, tag):
                halves = []
                for hh in range(2):
                    t = xinp.tile([128, 4, SL], BF16, tag="x",
                                  name=f"x{tag}{n}_{hh}")
                    nc.sync.dma_start(
                        t[:, :, :],
                        src[hh * 512:(hh + 1) * 512,
                            n * SL:(n + 1) * SL].rearrange(
                                "(c p) q -> p c q", p=128))
                    halves.append(t)
                return halves

            def emit_qk_unit(n, xt, w_s, dst, b_s, m, hh, psh):
                hhs = (0, 1) if hh is None else (hh,)
                if hhs[0] == 0:
                    psh[m] = psG.tile([128, 512], F32, tag="g", bufs=2,
                                      name=f"qk{n}_{m}")
                ps = psh[m]
                for h2 in hhs:
                    for c in range(4):
                        mm(ps[:, :], w_s[:, h2 * 4 + c, m * 128:(m + 1) * 128],
                           xt[h2][:, c, :],
                           start=(h2 == 0 and c == 0),
                           stop=(h2 == 1 and c == 3))
                if hhs[-1] == 1:
                    del psh[m]
                    nc.vector.tensor_scalar_add(
                        dst[:, m, n * SL:(n + 1) * SL], ps[:, :],
                        b_s[:, m:m + 1])

            def emit_v_unit(n, xt, tt, hf, hh, psh):
                hhs = (0, 1) if hh is None else (hh,)
                if hhs[0] == 0:
                    psh[(tt, hf)] = psG.tile([128, 512], F32, tag="g", bufs=2,
                                             name=f"v{n}_{tt}_{hf}")
                ps = psh[(tt, hf)]
                for h2 in hhs:
                    for c in range(4):
                        mm(ps[:, 0:260], xt[h2][:, c, tt * 128:(tt + 1) * 128],
                           wv_s[:, h2 * 4 + c, hf * 260:(hf + 1) * 260],
                           start=(h2 == 0 and c == 0),
                           stop=(h2 == 1 and c == 3))
                if hhs[-1] == 1:
                    del psh[(tt, hf)]
                    kb = n * 4 + tt
                    nc.vector.tensor_add(
                        vh_s[:, kb, hf, :],
                        ps[:, 0:260], vb_s[:, hf * 260:(hf + 1) * 260])

            def emit_c_unit(n, tt, n2, ctxT_n):
                ps = psG.tile([128, 512], F32, tag="g", bufs=2, name=f"c{n}_{tt}_{n2}")
                for c in range(4):
                    mm(ps[:, :], ctxT_n[c][:, tt * 128:(tt + 1) * 128],
                       wo_s[:, c, n2 * 512:(n2 + 1) * 512],
                       start=(c == 0), stop=(c == 3))
                ot = outsp.tile([128, 512], BF16, tag="ot", name=f"ot{n}_{tt}_{n2}")
                nc.vector.tensor_copy(ot[:, :], ps[:, :])
                nc.sync.dma_start(
                    outp[(n * 4 + tt) * 128:(n * 4 + tt + 1) * 128,
                         n2 * 512:(n2 + 1) * 512], ot[:, :])


            def c_units(n, ctxT_n):
                units = []
                for tt in range(4):
                    for n2 in range(2):
                        units.append(lambda n=n, tt=tt, n2=n2: emit_c_unit(
                            n, tt, n2, ctxT_n))
                return units

            # ---------- slice 0 projections (prologue, no filler) ----------
            xts0 = {}
            emit_w_dmas("q")
            xts0["q"] = emit_x_dmas(0, xq, "q")
            emit_w_dmas("k")
            xts0["k"] = emit_x_dmas(0, xk, "k")
            emit_w_dmas("first")
            psh0 = {}
            for m in range(4):
                for hh in range(2):
                    emit_qk_unit(0, xts0["q"], wq_s, qt_s, bq_s, m, hh, psh0)
            emit_w_dmas("v")
            xts0["v"] = emit_x_dmas(0, xv, "v")
            for m in range(4):
                for hh in range(2):
                    emit_qk_unit(0, xts0["k"], wk_s, kt_s, bk_s, m, hh, psh0)
            emit_w_dmas("rest")
            for tt in range(4):
                for hf in range(2):
                    emit_v_unit(0, xts0["v"], tt, hf, None, psh0)

            # ---------- attention + pipeline ----------
            ctxT_all = {}   # n -> list of 4 chunk tiles

            for n in range(NS):
                # filler queue: C(n-1) first, then proj(n+1)
                fillers = []
                if n >= 1:
                    fillers += c_units(n - 1, ctxT_all[n - 1])
                if n + 1 < NS:
                    # lazy x DMAs: emit per-tensor right before first use
                    pending_dma = {"q": xq, "k": xk, "v": xv}
                    nxts = {}
                    pshn = {}

                    def get_xt(which, n1=n + 1):
                        if which in pending_dma:
                            nxts[which] = emit_x_dmas(n1, pending_dma.pop(which),
                                                      which)
                        return nxts[which]

                    for m in range(4):
                        fillers.append(
                            lambda n1=n + 1, m=m: emit_qk_unit(
                                n1, get_xt("q"), wq_s, qt_s, bq_s, m, None,
                                pshn))
                    for m in range(4):
                        fillers.append(
                            lambda n1=n + 1, m=m: emit_qk_unit(
                                n1, get_xt("k"), wk_s, kt_s, bk_s, m, None,
                                pshn))
                    for tt in range(4):
                        for hf in range(2):
                            fillers.append(
                                lambda n1=n + 1, tt=tt, hf=hf:
                                emit_v_unit(n1, get_xt("v"), tt, hf, None,
                                            pshn))
                fill_i = [0]

                def pop_fillers(force_all=False, force_n=0):
                    while fill_i[0] < len(fillers) and (
                            force_all or fill_i[0] < force_n
                            or st["pe"] < st["act"] + float(_os.environ.get("K_MARGIN", 2500))):
                        fillers[fill_i[0]]()
                        fill_i[0] += 1

                nkb = 4 * n + 4
                ngrp = nkb // 2
                ctxT_n = [ctxTp.tile([128, 512], BF16, tag="ctxT",
                                     name=f"ctxT{n}_{c}") for c in range(4)]
                ctxT_all[n] = ctxT_n
                psT_cur = [None]

                def emit_s_exp(h, g, pts):
                    # Both banks of a group share the group's column base so a
                    # single 2-bank exp covers them (the extra computed scores
                    # land in q-chunks the AV stage never reads).
                    mc, po = h // 2, (h % 2) * 64
                    sps = psS.tile([128, 2, 512], F32, tag="s",
                                   name=f"s{n}_{h}_{g}")
                    pt = ptp.tile([128, 2, 512], BF16, tag="pt",
                                  name=f"pt{n}_{h}_{g}")
                    c0a = max(0, 2 * g * 128 - n * SL)
                    for i in range(2):
                        kb = 2 * g + i
                        mm(sps[:, i, c0a:],
                           kt_s[po:po + 64, mc, kb * 128:(kb + 1) * 128],
                           qt_s[po:po + 64, mc, n * SL + c0a:(n + 1) * SL],
                           start=True, stop=True)
                    st["act"] += (2 * (512 - c0a)) * ACT_NS + 185.0
                    nc.scalar.activation(
                        pt[:, :, c0a:], sps[:, :, c0a:],
                        func=mybir.ActivationFunctionType.Exp)
                    pts[(h, g)] = pt

                def emit_av(h, g, avp, pts, first_grp, last_grp):
                    # PSUM start=True lazily zero-marks the WHOLE bank, so
                    # only the first emitted write into the bank may use it;
                    # later first-writes per region overwrite via the
                    # pending-zero flags.  Accumulation order over kb is free.
                    pt = pts.pop((h, g))
                    for i in range(2):
                        kb = 2 * g + i
                        if kb >= 4 * n:  # diagonal block: causal mask
                            col0 = max(0, kb * 128 - n * SL)
                            (nc.gpsimd if _os.environ.get("K_MASKPOOL")
                             else nc.vector).tensor_mul(
                                pt[:, i, col0:col0 + 128],
                                pt[:, i, col0:col0 + 128], msk_s[:, :])
                    started = [not (g == first_grp)]
                    for i in range(2):
                        kb = 2 * g + i
                        for qc in range(4):
                            if kb > 4 * n + qc:
                                continue
                            last = (g == last_grp) and (
                                kb == min(1, 4 * n + qc))
                            mm(avp[:, qc, 0:65],
                               pt[:, i, qc * 128:(qc + 1) * 128],
                               vh_s[:, kb, h // 4, (h % 4) * 65:(h % 4) * 65 + 65],
                               start=not started[0],
                               stop=last,
                               skip_group_check=True)
                            started[0] = True

                def emit_tail(h, avp):
                    # normalize: ctx_t[q, d] = av[q, d] / av[q, 64]
                    mc, po = h // 2, (h % 2) * 64
                    rcp = smallp.tile([128, 4, 1], F32, tag="rcp",
                                      name=f"rcp{n}_{h}")
                    nc.vector.reciprocal(rcp[:, :], avp[:, :, 64:65])
                    ctx_t = ctxp.tile([128, 4, DK], BF16, tag="ctx",
                                      name=f"ctx{n}_{h}")
                    for qc in range(4):
                        nc.vector.tensor_scalar_mul(
                            ctx_t[:, qc, :], avp[:, qc, 0:DK], rcp[:, qc, :])
                    # transpose to dims-major; 2 heads share one psum tile
                    if h % 2 == 0:
                        psT_cur[0] = psG.tile([128, 512], BF16, tag="t",
                                              name=f"t{n}_{mc}")
                    psT = psT_cur[0]
                    for qc in range(4):
                        st["pe"] += 128 * PE_NS
                        nc.tensor.transpose(
                            psT[po:po + 64, qc * 128:(qc + 1) * 128],
                            ctx_t[:, qc, :], idn_s[:, :])
                    if h % 2 == 1:
                        nc.vector.tensor_copy(ctxT_n[mc][:, :], psT[:, :])

                # flat (head, group) pipeline: AV lags S/exp by one item so
                # head boundaries don't bunch the Act queue against psS WARs
                gorder = list(range(ngrp - 1, -1, -1))  # diag groups first
                items = [(h, g) for h in range(HD) for g in gorder]
                pts = {}
                avps = {}
                pending = []

                def drain_av():
                    ph, pg = pending.pop(0)
                    emit_av(ph, pg, avps[ph], pts, gorder[0], gorder[-1])
                    if pg == gorder[-1]:
                        emit_tail(ph, avps.pop(ph))

                for (h, g) in items:
                    if g == gorder[0]:
                        avps[h] = psAV.tile([128, 4, 128], F32, tag="av",
                                            name=f"av{n}_{h}")
                    emit_s_exp(h, g, pts)
                    pending.append((h, g))
                    if len(pending) > 2:
                        drain_av()
                    pop_fillers()
                while pending:
                    drain_av()

                pop_fillers(force_all=True)
                if dbg:
                    for c in range(4):
                        ct = outsp.tile([128, 512], F32, tag="dbg",
                                        name=f"cdb{n}_{c}")
                        nc.vector.tensor_copy(ct[:, :], ctxT_n[c][:, :])
                        nc.sync.dma_start(
                            ctx_dbg[:, (n * 4 + c) * 512:(n * 4 + c + 1) * 512],
                            ct[:, :])

            # ---------- final output projection ----------
            for u in c_units(NS - 1, ctxT_all[NS - 1]):
                u()

            if dbg:
                nc.sync.dma_start(qt_dbg[:, :], qt_s[:, :, :].bitcast(F32))
                nc.sync.dma_start(kt_dbg[:, :], kt_s[:, :, :].bitcast(F32))
                for kb in range(NKB):
                    for hf in range(2):
                        vt = outsp.tile([128, 512], F32, tag="dbg",
                                        name=f"vdb{kb}_{hf}")
                        nc.vector.tensor_copy(vt[:, 0:260], vh_s[:, kb, hf, :])
                        nc.sync.dma_start(
                            vh_dbg[:, kb * 520 + hf * 260:
                                   kb * 520 + (hf + 1) * 260], vt[:, 0:260])

    nc.compile()
    return nc


_NC = None
LAST_RESULTS = None


def kernel(**inputs):
    global _NC, LAST_RESULTS
    import os
    import ml_dtypes
    if _NC is None:
        _NC = _build_nc()

    f = lambda a: np.asarray(a, dtype=np.float32)
    q, k, v = f(inputs["q"]), f(inputs["k"]), f(inputs["v"])
    wq_w, wq_b = f(inputs["wq_w"]), f(inputs["wq_b"])
    wk_w, wk_b = f(inputs["wk_w"]), f(inputs["wk_b"])
    wv_w, wv_b = f(inputs["wv_w"]), f(inputs["wv_b"])
    wo_w, wo_b = f(inputs["wo_w"]), f(inputs["wo_b"])

    bf = ml_dtypes.bfloat16
    msk = np.ascontiguousarray(
        (np.arange(128)[None, :] >= np.arange(128)[:, None])).astype(bf)
    idn = np.eye(128).astype(bf)

    gmaps = []
    for g in range(2):
        sl = slice(g * GW, (g + 1) * GW)
        wqT = np.ascontiguousarray((wq_w[sl] * 0.125).T).astype(bf)
        wkT = np.ascontiguousarray(wk_w[sl].T).astype(bf)
        wvT = np.zeros((D, AUGW), np.float32)
        vbias = np.zeros((AUGW,), np.float32)
        for h in range(HD):
            wvT[:, h * 65:h * 65 + 64] = wv_w[g * GW + h * 64:
                                              g * GW + (h + 1) * 64].T
            vbias[h * 65:h * 65 + 64] = wv_b[g * GW + h * 64:
                                             g * GW + (h + 1) * 64]
            vbias[h * 65 + 64] = 1.0
        woT = np.ascontiguousarray(wo_w[:, sl].T).astype(bf)
        bqT = np.ascontiguousarray((wq_b[sl] * 0.125).reshape(4, 128).T)
        bkT = np.ascontiguousarray(wk_b[sl].reshape(4, 128).T)
        gmaps.append(dict(wq=wqT, wk=wkT, wv=wvT.astype(bf), wo=woT, bq=bqT, bk=bkT,
                          vb=vbias, msk=msk, idn=idn))

    bmaps = []
    for b in range(B):
        bmaps.append(dict(
            xq=np.ascontiguousarray(q[b].T).astype(bf),
            xk=np.ascontiguousarray(k[b].T).astype(bf),
            xv=np.ascontiguousarray(v[b].T).astype(bf)))

    in_maps = [dict(**bmaps[c // 2], **gmaps[c % 2]) for c in range(8)]

    trace = bool(int(os.environ.get("KERNEL_TRACE", "0")))
    res = run_bass_kernel_spmd(_NC, in_maps, list(range(8)), trace=trace)
    LAST_RESULTS = res

    out = np.empty((B, L, D), np.float32)
    for b in range(B):
        out[b] = (np.asarray(res.results[2 * b]["outp"], np.float32)
                  + np.asarray(res.results[2 * b + 1]["outp"], np.float32)
                  + wo_b[None, :])
    return out
